# revision 48
# baseline (speedup 1.0000x reference)
"""Trainium2 Bass kernel for nn_MHSA_37821482008969 (2D rel-pos MHSA).

Strategy: data-parallel over batch (16 batches -> 8 cores x 2). Per (batch,
head) unit, attention is computed fully transposed: S^T = K^T@Q tiles with
y (keys) on partitions, so softmax-normalization sums come from a ones-matrix
matmul on PE (replicated across all 128 partitions, so the reciprocal and
final scale run as plain full-width DVE ops), the attn matmul needs no
transposes of exp(S), and the output lands directly in the channel-major
layout the conv output wants.

Rel-pos biases are folded into the logits accumulation as one extra K=64
matmul per tile: lhsT is a constant 0/1 selector, rhs is the skewed rel-logit
table built via a DRAM round-trip (regular strided APs implement the
rel->abs skew) plus small per-j-block PE transposes for the width term.

All matmul operands are bf16 (fp32 PSUM accumulation); softmax skips the
row-max subtraction (logits are ~N(0,1), |logit| < 7, exp is safe in fp32).

The emission order software-pipelines the attention units (S-matmuls of unit
i+1 before the normalization tail of unit i) and interleaves batch 1's
projection/bias phase into batch 0's attention phase so the PE never idles
(keeps the tensor engine p-state at max clock).
"""
import numpy as np
import ml_dtypes

import concourse.bass as bass
import concourse.mybir as mybir
import concourse.tile as tile
import concourse.bacc as bacc
from concourse.bass_utils import run_bass_kernel_spmd

bf16 = ml_dtypes.bfloat16
f8e4 = ml_dtypes.float8_e4m3
FP32 = mybir.dt.float32
BF16 = mybir.dt.bfloat16
F8E4 = mybir.dt.float8e4
DR = mybir.MatmulPerfMode.DoubleRow

HEADS, D, F, DIM = 4, 128, 32, 512
L = F * F           # 1024
B_PER_CORE = 2
N_CORES = 8
AF = mybir.ActivationFunctionType

_cache = {}


def _build():
    nc = bacc.Bacc("TRN2", target_bir_lowering=False, debug=False,
                   num_devices=N_CORES)
    # host-packed layouts (see _prep_inputs)
    xin = nc.dram_tensor("xin", [B_PER_CORE, 8, 128, 4 * 128], BF16,
                         kind="ExternalInput").ap()
    wqt = nc.dram_tensor("wqt", [128, 4 * DIM], BF16, kind="ExternalInput").ap()
    wkt = nc.dram_tensor("wkt", [128, 4 * DIM], BF16, kind="ExternalInput").ap()
    wvt = nc.dram_tensor("wvt", [128, 4 * DIM], BF16, kind="ExternalInput").ap()
    relwt = nc.dram_tensor("relwt", [128, 64], BF16, kind="ExternalInput").ap()
    relht = nc.dram_tensor("relht", [128, 64], BF16, kind="ExternalInput").ap()
    sel = nc.dram_tensor("sel", [64, 8 * 128], BF16, kind="ExternalInput").ap()
    ones = nc.dram_tensor("ones", [128, 128], BF16, kind="ExternalInput").ap()
    ident = nc.dram_tensor("ident", [128, 128], BF16, kind="ExternalInput").ap()
    out = nc.dram_tensor("out", [B_PER_CORE, DIM, L], FP32, kind="ExternalOutput").ap()

    from contextlib import ExitStack
    ctx = ExitStack()
    with tile.TileContext(nc) as tc, ctx:
        consts = ctx.enter_context(tc.tile_pool(name="consts", bufs=1))
        xpool = ctx.enter_context(tc.tile_pool(name="xpool", bufs=2))
        vtpool = ctx.enter_context(tc.tile_pool(name="vtpool", bufs=2))
        qkpool = ctx.enter_context(tc.tile_pool(name="qkpool", bufs=2))
        rwpool = ctx.enter_context(tc.tile_pool(name="rwpool", bufs=2))
        biaspool = ctx.enter_context(tc.tile_pool(name="biaspool", bufs=2))
        ptpool = ctx.enter_context(tc.tile_pool(name="ptpool", bufs=3))
        outpool = ctx.enter_context(tc.tile_pool(name="outpool", bufs=2))
        # PSUM: 8 banks total: st 3 + attn 2 + sums 1 + misc 2
        stps = ctx.enter_context(tc.tile_pool(name="stps", bufs=3, space="PSUM"))
        attnps = ctx.enter_context(tc.tile_pool(name="attnps", bufs=2, space="PSUM"))
        sumsps = ctx.enter_context(tc.tile_pool(name="sumsps", bufs=1, space="PSUM"))
        miscps = ctx.enter_context(tc.tile_pool(name="miscps", bufs=2, space="PSUM"))
        dramw = ctx.enter_context(tc.tile_pool(name="dramw", bufs=2, space="DRAM"))
        dramh = ctx.enter_context(tc.tile_pool(name="dramh", bufs=2, space="DRAM"))

        # ---- constants ride the scalar HWDGE queue so they issue in
        # parallel with x/bounce/output traffic on the sync queue ----
        def cload(ap, shape, tag):
            t = consts.tile(shape, ap.dtype, tag=tag)
            nc.scalar.dma_start(t[:], ap)
            return t

        x_sb = [None, None]

        def emit_xload(b):
            # x SBUF free layout is (yt, c, 128): per-yt loads are contiguous
            # on both sides, so V-proj can start after the first fast 128KB
            x_sb[b] = xpool.tile([128, 8, 4, 128], BF16, tag="x", name=f"x{b}")
            for yt in range(8):
                nc.sync.dma_start(x_sb[b][:, yt], xin[b, yt])

        wv_sb = cload(wvt, [128, 4 * DIM], "wv")
        emit_xload(0)
        wq_sb = cload(wqt, [128, 4 * DIM], "wq")
        wk_sb = cload(wkt, [128, 4 * DIM], "wk")
        relw_sb = cload(relwt, [128, 64], "relw")
        relh_sb = cload(relht, [128, 64], "relh")
        id_sb = cload(ident, [128, 128], "ident")
        sel_sb = cload(sel, [64, 8 * 128], "sel")
        ones_sb = cload(ones, [128, 128], "ones")

        # per-batch state
        vt_sb = [[None] * 8, [None] * 8]
        qs = [[None] * HEADS, [None] * HEADS]
        ks = [[None] * HEADS, [None] * HEADS]
        biases = [[None] * HEADS, [None] * HEADS]

        def xc(b, c, lo, hi):
            # x chunk c, L-columns [lo:hi) (lo/hi multiples of 128)
            return x_sb[b][:, lo // 128: hi // 128, c, :]

        def emit_vproj(b, yts):
            for yt in yts:
                ps = miscps.tile([128, DIM], FP32, tag="misc")
                for c in range(4):
                    nc.tensor.matmul(ps[:], xc(b, c, yt * 128, (yt + 1) * 128),
                                     wv_sb[:, c * DIM:(c + 1) * DIM],
                                     start=(c == 0), stop=(c == 3))
                vt = vtpool.tile([128, DIM], BF16, tag=f"vt{yt}")
                nc.vector.tensor_copy(vt[:], ps[:])
                vt_sb[b][yt] = vt

        def emit_qkproj(b, h):
            q_sb = qkpool.tile([128, L], BF16, tag=f"q{h}")
            k_sb = qkpool.tile([128, L], BF16, tag=f"k{h}")
            qs[b][h] = q_sb
            ks[b][h] = k_sb
            for dst, w in ((q_sb, wq_sb), (k_sb, wk_sb)):
                pss = [miscps.tile([128, 512], FP32, tag="misc", name=f"qkps{i}")
                       for i in range(2)]
                for c in range(4):
                    for n in range(2):
                        nc.tensor.matmul(pss[n][:],
                                         w[:, c * DIM + h * 128: c * DIM + (h + 1) * 128],
                                         xc(b, c, n * 512, (n + 1) * 512),
                                         start=(c == 0), stop=(c == 3))
                for n in range(2):
                    nc.vector.tensor_copy(dst[:, n * 512:(n + 1) * 512], pss[n][:])

        wst_tiles = [[None] * 8, [None] * 8]

        def emit_rel_a(b, h):
            # matmuls + DRAM-bounce DMA issues; no PE instruction here waits
            # on the bounce, so the PE queue never stalls on DMA latency
            q_sb = qs[b][h]
            bias_rhs = biaspool.tile([64, L], BF16, tag=f"bias{h}")
            biases[b][h] = bias_rhs
            # ---- raw width logits rl[q, m] = q . relw, all 8 q-blocks in 1 bank
            rwps = miscps.tile([128, 512], FP32, tag="misc")
            for j in range(8):
                nc.tensor.matmul(rwps[:, j * 64:(j + 1) * 64],
                                 q_sb[:, j * 128:(j + 1) * 128], relw_sb[:],
                                 start=True, stop=True)
            rwall = rwpool.tile([128, 512], BF16, tag="rwall")
            nc.vector.tensor_copy(rwall[:], rwps[:])
            # bounce to DRAM in [q, m] row-major order (row stride 64);
            # the whole bounce rides the idle gpsimd SWDGE queue so it never
            # contends with bulk loads/stores on the sync queue
            skw = dramw.tile([L, 64], BF16, tag="skw")
            skw_flat = skw[:].flatten()
            dst = bass.AP(skw_flat.tensor, skw_flat.offset,
                          [[64, 128], [8192, 8], [1, 64]])
            nc.sync.dma_start(dst, rwall[:])
            # ---- skewed reads per 128-wide x-block j ----
            for j in range(8):
                wstj = rwpool.tile([128, 32], BF16, tag=f"wst{j}", name=f"wst{j}")
                src = bass.AP(skw_flat.tensor,
                              skw_flat.offset + 8192 * j + 31,
                              [[2048, 4], [63, 32], [1, 32]])
                nc.sync.dma_start(wstj[:], src)
                wst_tiles[b][j] = wstj
            # ---- height logits rl_h[m, q] (relht col 63 zero-padded) ----
            rh = rwpool.tile([64, L], BF16, tag="rh")
            for n in range(2):
                ps = miscps.tile([64, 512], FP32, tag="misc")
                nc.tensor.matmul(ps[:], relh_sb[:],
                                 q_sb[:, n * 512:(n + 1) * 512],
                                 start=True, stop=True)
                nc.vector.tensor_copy(rh[:, n * 512:(n + 1) * 512], ps[:])
            skh = dramh.tile([64, L], BF16, tag="skh")
            nc.sync.dma_start(skh[:], rh[:])
            skh_flat = skh[:].flatten()
            hsrc = bass.AP(skh_flat.tensor, skh_flat.offset,
                           [[1024, 32], [1056, 32], [1, 32]])
            bias_flat = bias_rhs[:]
            hdst = bass.AP(bias_flat.tensor, bias_flat.offset + 32 * 1024,
                           [[1024, 32], [32, 32], [1, 32]])
            nc.sync.dma_start(hdst, hsrc)

        def emit_rel_b(b, h):
            # transposes of the skew-read tiles into bias_rhs rows [0:32)
            bias_rhs = biases[b][h]
            for half in range(2):
                tps = miscps.tile([32, 512], BF16, tag="misc")
                for jj in range(4):
                    j = half * 4 + jj
                    nc.tensor.transpose(tps[0:32, jj * 128:(jj + 1) * 128],
                                        wst_tiles[b][j][:], id_sb[:])
                nc.vector.tensor_copy(
                    bias_rhs[0:32, half * 512:(half + 1) * 512], tps[0:32, :])

        def emit_S(b, h, n, tree=True):
            q_sb, k_sb, bias_rhs = qs[b][h], ks[b][h], biases[b][h]
            pts = []
            for yt in range(8):
                st = stps.tile([128, 512], FP32, tag="st")
                nc.tensor.matmul(st[:], k_sb[:, yt * 128:(yt + 1) * 128],
                                 q_sb[:, n * 512:(n + 1) * 512],
                                 start=True, stop=False)
                nc.tensor.matmul(st[:], sel_sb[:, yt * 128:(yt + 1) * 128],
                                 bias_rhs[:, n * 512:(n + 1) * 512],
                                 start=False, stop=True)
                pt = ptpool.tile([128, 512], BF16, tag=f"pt{yt}")
                nc.scalar.activation(pt[:], st[:], AF.Exp)
                pts.append(pt)
            if not tree:
                return pts, None
            # DVE pairwise tree for the softmax denominator: the PE then only
            # streams one ones-matmul per unit instead of 8 (bf16 ops hit the
            # DVE 2x mode; final bf16 rounding costs ~0.2% on the denominator)
            l1 = [ptpool.tile([128, 512], BF16, tag=f"l1{i}", name=f"l1{i}")
                  for i in range(4)]
            for i in range(4):
                nc.vector.tensor_add(l1[i][:], pts[2 * i][:], pts[2 * i + 1][:])
            l2 = [ptpool.tile([128, 512], BF16, tag=f"l2{i}", name=f"l2{i}")
                  for i in range(2)]
            for i in range(2):
                nc.vector.tensor_add(l2[i][:], l1[2 * i][:], l1[2 * i + 1][:])
            psum_all = ptpool.tile([128, 512], BF16, tag="psall")
            nc.vector.tensor_add(psum_all[:], l2[0][:], l2[1][:])
            return pts, psum_all

        def emit_tail(b, h, n, pts, psum_all, split=False):
            sums = sumsps.tile([128, 512], FP32, tag="sums")
            attn = attnps.tile([128, 512], FP32, tag="attn")
            if psum_all is None:
                # last unit: PE-sums before attn, so the reciprocal overlaps
                # the attn matmuls and the final-store chain is short
                for yt in range(8):
                    nc.tensor.matmul(sums[:], ones_sb[:], pts[yt][:],
                                     start=(yt == 0), stop=(yt == 7))
            for yt in range(8):
                nc.tensor.matmul(attn[:], vt_sb[b][yt][:, h * 128:(h + 1) * 128],
                                 pts[yt][:], start=(yt == 0), stop=(yt == 7))
            if psum_all is not None:
                nc.tensor.matmul(sums[:], ones_sb[:], psum_all[:],
                                 start=True, stop=True)
            recip = outpool.tile([128, 512], FP32, tag="recip")
            o_sb = outpool.tile([128, 512], FP32, tag="osb")
            # split halves on the last unit so the final store starts sooner
            for lo, hi in ([(0, 256), (256, 512)] if split else [(0, 512)]):
                nc.vector.reciprocal_approx_fast(out=recip[:, lo:hi],
                                                 in_=sums[:, lo:hi])
                nc.vector.tensor_mul(o_sb[:, lo:hi], attn[:, lo:hi],
                                     recip[:, lo:hi])
                nc.sync.dma_start(
                    out[b, h * 128:(h + 1) * 128, n * 512 + lo: n * 512 + hi],
                    o_sb[:, lo:hi])

        # ---- emission schedule ----
        # per batch: [V01, V23, QK0, RELa0, QK1, RELb0, RELa1, QK2, RELb1,
        #             RELa2, QK3, RELb2, RELa3, RELb3]; a unit (b, h) needs
        # thunks through RELb(h) => req offsets {0: 6, 1: 9, 2: 12, 3: 14}
        def phase1_thunks(b):
            ts = [lambda b=b: emit_vproj(b, range(0, 4)),
                  lambda b=b: emit_vproj(b, range(4, 8)),
                  lambda b=b: emit_qkproj(b, 0),
                  lambda b=b: emit_rel_a(b, 0)]
            for h in range(1, HEADS):
                ts += [lambda b=b, h=h: emit_qkproj(b, h),
                       lambda b=b, h=h: emit_rel_b(b, h - 1),
                       lambda b=b, h=h: emit_rel_a(b, h)]
            ts += [lambda b=b: emit_rel_b(b, HEADS - 1)]
            return ts

        REQ = {0: 6, 1: 9, 2: 12, 3: 14}
        thunks = (phase1_thunks(0) +
                  [lambda: emit_xload(1)] + phase1_thunks(1))
        n_consumed = 0

        def consume(upto=None, extra=0):
            nonlocal n_consumed
            target = n_consumed + extra if upto is None else max(upto, n_consumed)
            target = min(target, len(thunks))
            while n_consumed < target:
                thunks[n_consumed]()
                n_consumed += 1

        units = [(b, h, n) for b in range(B_PER_CORE)
                 for h in range(HEADS) for n in range(2)]
        from collections import deque
        pending = deque()
        consume(upto=7)   # fill the cold-start bounce latency with PE work
        for ui, (b, h, n) in enumerate(units):
            consume(upto=15 * b + REQ[h])   # need V + QK/REL through head h
            pts, psum_all = emit_S(b, h, n, tree=(ui < len(units) - 1))
            consume(extra=1)
            # 2-unit skew: deps (exp, tree) of a tail are long complete by
            # the time the PE reaches it, so matmuls never eat SEM_DELAY
            if len(pending) == 2:
                emit_tail(*pending.popleft())
            consume(extra=1)
            pending.append((b, h, n, pts, psum_all))
        while pending:
            emit_tail(*pending.popleft())

    nc.compile()
    return nc


def _prep_inputs(featuremap, w_qk, w_v, rel_height, rel_width):
    scale = D ** -0.5
    # weights packed as [128, c_chunk*512]: w[p, c*512+d] = W.T[c*128+p, d]
    def packw(wt):  # wt: [512(c), 512(d)]
        return np.ascontiguousarray(
            wt.reshape(4, 128, DIM).transpose(1, 0, 2).reshape(128, 4 * DIM)
        ).astype(bf16)
    wqt = packw(w_qk[:DIM].T * scale)
    wkt = packw(w_qk[DIM:].T)
    wvt = packw(w_v.T)
    relwt = np.zeros((128, 64), np.float32)
    relwt[:, :63] = rel_width.T
    relwt = relwt.astype(bf16)
    relht = np.zeros((128, 64), np.float32)
    relht[:, :63] = rel_height.T[:, ::-1]
    relht = relht.astype(bf16)
    yy = np.arange(128)
    sel = np.zeros((64, 8 * 128), np.float32)
    for yt in range(8):
        sel[yy % 32, yt * 128 + yy] = 1.0
        sel[32 + 31 - (yt * 4 + yy // 32), yt * 128 + yy] = 1.0
    sel = sel.astype(bf16)
    ones = np.ones((128, 128), bf16)
    ident = np.eye(128, dtype=bf16)
    common = dict(wqt=wqt, wkt=wkt, wvt=wvt, relwt=relwt, relht=relht,
                  sel=sel, ones=ones, ident=ident)
    # x packed per batch as [8(yt), 128(p), 4(c), 128(l)] (yt-major, so each
    # per-yt DMA is contiguous on both sides)
    xin = featuremap.reshape(16, 4, 128, 8, 128).transpose(0, 3, 2, 1, 4).reshape(
        N_CORES, B_PER_CORE, 8, 128, 4 * 128).astype(bf16)
    return [dict(common, xin=np.ascontiguousarray(xin[i])) for i in range(N_CORES)]


def kernel(featuremap, w_qk, w_v, rel_height, rel_width, _trace=False, _tmpdir=None):
    if "nc" not in _cache:
        _cache["nc"] = _build()
    nc = _cache["nc"]
    in_maps = _prep_inputs(featuremap, w_qk, w_v, rel_height, rel_width)
    res = run_bass_kernel_spmd(nc, in_maps, list(range(N_CORES)),
                               trace=_trace, tmpdir=_tmpdir)
    _cache["last_result"] = res
    full = np.concatenate([res.results[i]["out"] for i in range(N_CORES)], axis=0)
    return full.reshape(16, DIM, F, F)


# revision 50
# speedup vs baseline: 1.0055x; 1.0055x over previous
"""Trainium2 Bass kernel for nn_MHSA_37821482008969 (2D rel-pos MHSA).

Strategy: data-parallel over batch (16 batches -> 8 cores x 2). Per (batch,
head) unit, attention is computed fully transposed: S^T = K^T@Q tiles with
y (keys) on partitions, so softmax-normalization sums come from a ones-matrix
matmul on PE (replicated across all 128 partitions, so the reciprocal and
final scale run as plain full-width DVE ops), the attn matmul needs no
transposes of exp(S), and the output lands directly in the channel-major
layout the conv output wants.

Rel-pos biases are folded into the logits accumulation as one extra K=64
matmul per tile: lhsT is a constant 0/1 selector, rhs is the skewed rel-logit
table built via a DRAM round-trip (regular strided APs implement the
rel->abs skew) plus small per-j-block PE transposes for the width term.

All matmul operands are bf16 (fp32 PSUM accumulation); softmax skips the
row-max subtraction (logits are ~N(0,1), |logit| < 7, exp is safe in fp32).

The emission order software-pipelines the attention units (S-matmuls of unit
i+1 before the normalization tail of unit i) and interleaves batch 1's
projection/bias phase into batch 0's attention phase so the PE never idles
(keeps the tensor engine p-state at max clock).
"""
import numpy as np
import ml_dtypes

import concourse.bass as bass
import concourse.mybir as mybir
import concourse.tile as tile
import concourse.bacc as bacc
from concourse.bass_utils import run_bass_kernel_spmd

bf16 = ml_dtypes.bfloat16
f8e4 = ml_dtypes.float8_e4m3
FP32 = mybir.dt.float32
BF16 = mybir.dt.bfloat16
F8E4 = mybir.dt.float8e4
DR = mybir.MatmulPerfMode.DoubleRow

HEADS, D, F, DIM = 4, 128, 32, 512
L = F * F           # 1024
B_PER_CORE = 2
N_CORES = 8
AF = mybir.ActivationFunctionType

_cache = {}


def _build():
    nc = bacc.Bacc("TRN2", target_bir_lowering=False, debug=False,
                   num_devices=N_CORES)
    # host-packed layouts (see _prep_inputs)
    xin = nc.dram_tensor("xin", [B_PER_CORE, 8, 128, 4 * 128], BF16,
                         kind="ExternalInput").ap()
    wqt = nc.dram_tensor("wqt", [128, 4 * DIM], BF16, kind="ExternalInput").ap()
    wkt = nc.dram_tensor("wkt", [128, 4 * DIM], BF16, kind="ExternalInput").ap()
    wvt = nc.dram_tensor("wvt", [128, 4 * DIM], BF16, kind="ExternalInput").ap()
    relwt = nc.dram_tensor("relwt", [128, 64], BF16, kind="ExternalInput").ap()
    relht = nc.dram_tensor("relht", [128, 64], BF16, kind="ExternalInput").ap()
    sel = nc.dram_tensor("sel", [64, 8 * 128], BF16, kind="ExternalInput").ap()
    ones = nc.dram_tensor("ones", [128, 128], BF16, kind="ExternalInput").ap()
    ident = nc.dram_tensor("ident", [128, 128], BF16, kind="ExternalInput").ap()
    out = nc.dram_tensor("out", [B_PER_CORE, DIM, L], FP32, kind="ExternalOutput").ap()

    from contextlib import ExitStack
    ctx = ExitStack()
    with tile.TileContext(nc) as tc, ctx:
        consts = ctx.enter_context(tc.tile_pool(name="consts", bufs=1))
        xpool = ctx.enter_context(tc.tile_pool(name="xpool", bufs=2))
        vtpool = ctx.enter_context(tc.tile_pool(name="vtpool", bufs=2))
        qkpool = ctx.enter_context(tc.tile_pool(name="qkpool", bufs=2))
        rwpool = ctx.enter_context(tc.tile_pool(name="rwpool", bufs=2))
        biaspool = ctx.enter_context(tc.tile_pool(name="biaspool", bufs=2))
        ptpool = ctx.enter_context(tc.tile_pool(name="ptpool", bufs=2))
        outpool = ctx.enter_context(tc.tile_pool(name="outpool", bufs=2))
        # PSUM: 8 banks total: st 3 + attn 2 + sums 1 + misc 2
        stps = ctx.enter_context(tc.tile_pool(name="stps", bufs=3, space="PSUM"))
        attnps = ctx.enter_context(tc.tile_pool(name="attnps", bufs=2, space="PSUM"))
        sumsps = ctx.enter_context(tc.tile_pool(name="sumsps", bufs=1, space="PSUM"))
        miscps = ctx.enter_context(tc.tile_pool(name="miscps", bufs=2, space="PSUM"))
        dramw = ctx.enter_context(tc.tile_pool(name="dramw", bufs=2, space="DRAM"))
        dramh = ctx.enter_context(tc.tile_pool(name="dramh", bufs=2, space="DRAM"))

        # ---- constants ride the scalar HWDGE queue so they issue in
        # parallel with x/bounce/output traffic on the sync queue ----
        def cload(ap, shape, tag):
            t = consts.tile(shape, ap.dtype, tag=tag)
            nc.scalar.dma_start(t[:], ap)
            return t

        x_sb = [None, None]

        def emit_xload(b):
            # x SBUF free layout is (yt, c, 128): per-yt loads are contiguous
            # on both sides, so V-proj can start after the first fast 128KB
            x_sb[b] = xpool.tile([128, 8, 4, 128], BF16, tag="x", name=f"x{b}")
            for yt in range(8):
                nc.sync.dma_start(x_sb[b][:, yt], xin[b, yt])

        wv_sb = cload(wvt, [128, 4 * DIM], "wv")
        emit_xload(0)
        wq_sb = cload(wqt, [128, 4 * DIM], "wq")
        wk_sb = cload(wkt, [128, 4 * DIM], "wk")
        relw_sb = cload(relwt, [128, 64], "relw")
        relh_sb = cload(relht, [128, 64], "relh")
        id_sb = cload(ident, [128, 128], "ident")
        sel_sb = cload(sel, [64, 8 * 128], "sel")
        ones_sb = cload(ones, [128, 128], "ones")

        # per-batch state
        vt_sb = [[None] * 8, [None] * 8]
        qs = [[None] * HEADS, [None] * HEADS]
        ks = [[None] * HEADS, [None] * HEADS]
        biases = [[None] * HEADS, [None] * HEADS]

        def xc(b, c, lo, hi):
            # x chunk c, L-columns [lo:hi) (lo/hi multiples of 128)
            return x_sb[b][:, lo // 128: hi // 128, c, :]

        def emit_vproj(b, yts):
            for yt in yts:
                ps = miscps.tile([128, DIM], FP32, tag="misc")
                for c in range(4):
                    nc.tensor.matmul(ps[:], xc(b, c, yt * 128, (yt + 1) * 128),
                                     wv_sb[:, c * DIM:(c + 1) * DIM],
                                     start=(c == 0), stop=(c == 3))
                vt = vtpool.tile([128, DIM], BF16, tag=f"vt{yt}")
                nc.vector.tensor_copy(vt[:], ps[:])
                vt_sb[b][yt] = vt

        def emit_qkproj(b, h):
            q_sb = qkpool.tile([128, L], BF16, tag=f"q{h}")
            k_sb = qkpool.tile([128, L], BF16, tag=f"k{h}")
            qs[b][h] = q_sb
            ks[b][h] = k_sb
            for dst, w in ((q_sb, wq_sb), (k_sb, wk_sb)):
                pss = [miscps.tile([128, 512], FP32, tag="misc", name=f"qkps{i}")
                       for i in range(2)]
                for c in range(4):
                    for n in range(2):
                        nc.tensor.matmul(pss[n][:],
                                         w[:, c * DIM + h * 128: c * DIM + (h + 1) * 128],
                                         xc(b, c, n * 512, (n + 1) * 512),
                                         start=(c == 0), stop=(c == 3))
                for n in range(2):
                    nc.vector.tensor_copy(dst[:, n * 512:(n + 1) * 512], pss[n][:])

        wst_tiles = [[None] * 8, [None] * 8]

        def emit_rel_a(b, h):
            # matmuls + DRAM-bounce DMA issues; no PE instruction here waits
            # on the bounce, so the PE queue never stalls on DMA latency
            q_sb = qs[b][h]
            bias_rhs = biaspool.tile([64, L], BF16, tag=f"bias{h}")
            biases[b][h] = bias_rhs
            # ---- raw width logits rl[q, m] = q . relw, all 8 q-blocks in 1 bank
            rwps = miscps.tile([128, 512], FP32, tag="misc")
            for j in range(8):
                nc.tensor.matmul(rwps[:, j * 64:(j + 1) * 64],
                                 q_sb[:, j * 128:(j + 1) * 128], relw_sb[:],
                                 start=True, stop=True)
            rwall = rwpool.tile([128, 512], BF16, tag="rwall")
            nc.vector.tensor_copy(rwall[:], rwps[:])
            # bounce to DRAM in [q, m] row-major order (row stride 64);
            # the whole bounce rides the idle gpsimd SWDGE queue so it never
            # contends with bulk loads/stores on the sync queue
            skw = dramw.tile([L, 64], BF16, tag="skw")
            skw_flat = skw[:].flatten()
            dst = bass.AP(skw_flat.tensor, skw_flat.offset,
                          [[64, 128], [8192, 8], [1, 64]])
            nc.sync.dma_start(dst, rwall[:])
            # ---- skewed reads per 128-wide x-block j ----
            for j in range(8):
                wstj = rwpool.tile([128, 32], BF16, tag=f"wst{j}", name=f"wst{j}")
                src = bass.AP(skw_flat.tensor,
                              skw_flat.offset + 8192 * j + 31,
                              [[2048, 4], [63, 32], [1, 32]])
                nc.sync.dma_start(wstj[:], src)
                wst_tiles[b][j] = wstj
            # ---- height logits rl_h[m, q] (relht col 63 zero-padded) ----
            rh = rwpool.tile([64, L], BF16, tag="rh")
            for n in range(2):
                ps = miscps.tile([64, 512], FP32, tag="misc")
                nc.tensor.matmul(ps[:], relh_sb[:],
                                 q_sb[:, n * 512:(n + 1) * 512],
                                 start=True, stop=True)
                nc.vector.tensor_copy(rh[:, n * 512:(n + 1) * 512], ps[:])
            skh = dramh.tile([64, L], BF16, tag="skh")
            nc.sync.dma_start(skh[:], rh[:])
            skh_flat = skh[:].flatten()
            hsrc = bass.AP(skh_flat.tensor, skh_flat.offset,
                           [[1024, 32], [1056, 32], [1, 32]])
            bias_flat = bias_rhs[:]
            hdst = bass.AP(bias_flat.tensor, bias_flat.offset + 32 * 1024,
                           [[1024, 32], [32, 32], [1, 32]])
            nc.sync.dma_start(hdst, hsrc)

        def emit_rel_b(b, h):
            # transposes of the skew-read tiles into bias_rhs rows [0:32)
            bias_rhs = biases[b][h]
            for half in range(2):
                tps = miscps.tile([32, 512], BF16, tag="misc")
                for jj in range(4):
                    j = half * 4 + jj
                    nc.tensor.transpose(tps[0:32, jj * 128:(jj + 1) * 128],
                                        wst_tiles[b][j][:], id_sb[:])
                nc.vector.tensor_copy(
                    bias_rhs[0:32, half * 512:(half + 1) * 512], tps[0:32, :])

        def emit_S(b, h, n, tree=True):
            q_sb, k_sb, bias_rhs = qs[b][h], ks[b][h], biases[b][h]
            pts = []
            for yt in range(8):
                st = stps.tile([128, 512], FP32, tag="st")
                nc.tensor.matmul(st[:], k_sb[:, yt * 128:(yt + 1) * 128],
                                 q_sb[:, n * 512:(n + 1) * 512],
                                 start=True, stop=False)
                nc.tensor.matmul(st[:], sel_sb[:, yt * 128:(yt + 1) * 128],
                                 bias_rhs[:, n * 512:(n + 1) * 512],
                                 start=False, stop=True)
                pt = ptpool.tile([128, 512], BF16, tag=f"pt{yt}")
                nc.scalar.activation(pt[:], st[:], AF.Exp)
                pts.append(pt)
            if not tree:
                return pts, None
            # DVE pairwise tree for the softmax denominator: the PE then only
            # streams one ones-matmul per unit instead of 8 (bf16 ops hit the
            # DVE 2x mode; final bf16 rounding costs ~0.2% on the denominator)
            l1 = [ptpool.tile([128, 512], BF16, tag=f"l1{i}", name=f"l1{i}")
                  for i in range(4)]
            for i in range(4):
                nc.vector.tensor_add(l1[i][:], pts[2 * i][:], pts[2 * i + 1][:])
            l2 = [ptpool.tile([128, 512], BF16, tag=f"l2{i}", name=f"l2{i}")
                  for i in range(2)]
            for i in range(2):
                nc.vector.tensor_add(l2[i][:], l1[2 * i][:], l1[2 * i + 1][:])
            psum_all = ptpool.tile([128, 512], BF16, tag="psall")
            nc.vector.tensor_add(psum_all[:], l2[0][:], l2[1][:])
            return pts, psum_all

        def emit_tail(b, h, n, pts, psum_all, split=False):
            sums = sumsps.tile([128, 512], FP32, tag="sums")
            attn = attnps.tile([128, 512], FP32, tag="attn")
            if psum_all is None:
                # last unit: PE-sums before attn, so the reciprocal overlaps
                # the attn matmuls and the final-store chain is short
                for yt in range(8):
                    nc.tensor.matmul(sums[:], ones_sb[:], pts[yt][:],
                                     start=(yt == 0), stop=(yt == 7))
            for yt in range(8):
                nc.tensor.matmul(attn[:], vt_sb[b][yt][:, h * 128:(h + 1) * 128],
                                 pts[yt][:], start=(yt == 0), stop=(yt == 7))
            if psum_all is not None:
                nc.tensor.matmul(sums[:], ones_sb[:], psum_all[:],
                                 start=True, stop=True)
            recip = outpool.tile([128, 512], FP32, tag="recip")
            o_sb = outpool.tile([128, 512], FP32, tag="osb")
            # split halves on the last unit so the final store starts sooner
            for lo, hi in ([(0, 256), (256, 512)] if split else [(0, 512)]):
                nc.vector.reciprocal_approx_fast(out=recip[:, lo:hi],
                                                 in_=sums[:, lo:hi])
                nc.vector.tensor_mul(o_sb[:, lo:hi], attn[:, lo:hi],
                                     recip[:, lo:hi])
                nc.sync.dma_start(
                    out[b, h * 128:(h + 1) * 128, n * 512 + lo: n * 512 + hi],
                    o_sb[:, lo:hi])

        # ---- emission schedule ----
        # per batch: [V01, V23, QK0, RELa0, QK1, RELb0, RELa1, QK2, RELb1,
        #             RELa2, QK3, RELb2, RELa3, RELb3]; a unit (b, h) needs
        # thunks through RELb(h) => req offsets {0: 6, 1: 9, 2: 12, 3: 14}
        def phase1_thunks(b):
            ts = [lambda b=b: emit_vproj(b, range(0, 4)),
                  lambda b=b: emit_vproj(b, range(4, 8)),
                  lambda b=b: emit_qkproj(b, 0),
                  lambda b=b: emit_rel_a(b, 0)]
            for h in range(1, HEADS):
                ts += [lambda b=b, h=h: emit_qkproj(b, h),
                       lambda b=b, h=h: emit_rel_b(b, h - 1),
                       lambda b=b, h=h: emit_rel_a(b, h)]
            ts += [lambda b=b: emit_rel_b(b, HEADS - 1)]
            return ts

        REQ = {0: 6, 1: 9, 2: 12, 3: 14}
        thunks = (phase1_thunks(0) +
                  [lambda: emit_xload(1)] + phase1_thunks(1))
        n_consumed = 0

        def consume(upto=None, extra=0):
            nonlocal n_consumed
            target = n_consumed + extra if upto is None else max(upto, n_consumed)
            target = min(target, len(thunks))
            while n_consumed < target:
                thunks[n_consumed]()
                n_consumed += 1

        units = [(b, h, n) for b in range(B_PER_CORE)
                 for h in range(HEADS) for n in range(2)]
        from collections import deque
        pending = deque()
        consume(upto=7)   # fill the cold-start bounce latency with PE work
        for ui, (b, h, n) in enumerate(units):
            consume(upto=15 * b + REQ[h])   # need V + QK/REL through head h
            pts, psum_all = emit_S(b, h, n, tree=(ui < len(units) - 1))
            consume(extra=1)
            if len(pending) == 1:
                emit_tail(*pending.popleft())
            consume(extra=1)
            pending.append((b, h, n, pts, psum_all))
        while pending:
            emit_tail(*pending.popleft())

    nc.compile()
    return nc


def _prep_inputs(featuremap, w_qk, w_v, rel_height, rel_width):
    scale = D ** -0.5
    # weights packed as [128, c_chunk*512]: w[p, c*512+d] = W.T[c*128+p, d]
    def packw(wt):  # wt: [512(c), 512(d)]
        return np.ascontiguousarray(
            wt.reshape(4, 128, DIM).transpose(1, 0, 2).reshape(128, 4 * DIM)
        ).astype(bf16)
    wqt = packw(w_qk[:DIM].T * scale)
    wkt = packw(w_qk[DIM:].T)
    wvt = packw(w_v.T)
    relwt = np.zeros((128, 64), np.float32)
    relwt[:, :63] = rel_width.T
    relwt = relwt.astype(bf16)
    relht = np.zeros((128, 64), np.float32)
    relht[:, :63] = rel_height.T[:, ::-1]
    relht = relht.astype(bf16)
    yy = np.arange(128)
    sel = np.zeros((64, 8 * 128), np.float32)
    for yt in range(8):
        sel[yy % 32, yt * 128 + yy] = 1.0
        sel[32 + 31 - (yt * 4 + yy // 32), yt * 128 + yy] = 1.0
    sel = sel.astype(bf16)
    ones = np.ones((128, 128), bf16)
    ident = np.eye(128, dtype=bf16)
    common = dict(wqt=wqt, wkt=wkt, wvt=wvt, relwt=relwt, relht=relht,
                  sel=sel, ones=ones, ident=ident)
    # x packed per batch as [8(yt), 128(p), 4(c), 128(l)] (yt-major, so each
    # per-yt DMA is contiguous on both sides)
    xin = featuremap.reshape(16, 4, 128, 8, 128).transpose(0, 3, 2, 1, 4).reshape(
        N_CORES, B_PER_CORE, 8, 128, 4 * 128).astype(bf16)
    return [dict(common, xin=np.ascontiguousarray(xin[i])) for i in range(N_CORES)]


def kernel(featuremap, w_qk, w_v, rel_height, rel_width, _trace=False, _tmpdir=None):
    if "nc" not in _cache:
        _cache["nc"] = _build()
    nc = _cache["nc"]
    in_maps = _prep_inputs(featuremap, w_qk, w_v, rel_height, rel_width)
    res = run_bass_kernel_spmd(nc, in_maps, list(range(N_CORES)),
                               trace=_trace, tmpdir=_tmpdir)
    _cache["last_result"] = res
    full = np.concatenate([res.results[i]["out"] for i in range(N_CORES)], axis=0)
    return full.reshape(16, DIM, F, F)


# revision 54
# speedup vs baseline: 1.1458x; 1.1395x over previous
"""Trainium2 Bass kernel for nn_MHSA_37821482008969 (2D rel-pos MHSA).

Strategy: data-parallel over batch (16 batches -> 8 cores x 2). Per (batch,
head) unit, attention is computed fully transposed: S^T = K^T@Q tiles with
y (keys) on partitions, so softmax-normalization sums come from a ones-matrix
matmul on PE (replicated across all 128 partitions, so the reciprocal and
final scale run as plain full-width DVE ops), the attn matmul needs no
transposes of exp(S), and the output lands directly in the channel-major
layout the conv output wants.

Rel-pos biases are folded into the logits accumulation as one extra K=64
matmul per tile: lhsT is a constant 0/1 selector, rhs is the skewed rel-logit
table built via a DRAM round-trip (regular strided APs implement the
rel->abs skew) plus small per-j-block PE transposes for the width term.

All matmul operands are bf16 (fp32 PSUM accumulation); softmax skips the
row-max subtraction (logits are ~N(0,1), |logit| < 7, exp is safe in fp32).

The emission order software-pipelines the attention units (S-matmuls of unit
i+1 before the normalization tail of unit i) and interleaves batch 1's
projection/bias phase into batch 0's attention phase so the PE never idles
(keeps the tensor engine p-state at max clock).
"""
import numpy as np
import ml_dtypes

import concourse.bass as bass
import concourse.mybir as mybir
import concourse.tile as tile
import concourse.bacc as bacc
from concourse.bass_utils import run_bass_kernel_spmd

bf16 = ml_dtypes.bfloat16
f8e4 = ml_dtypes.float8_e4m3
FP32 = mybir.dt.float32
BF16 = mybir.dt.bfloat16
F8E4 = mybir.dt.float8e4
DR = mybir.MatmulPerfMode.DoubleRow

HEADS, D, F, DIM = 4, 128, 32, 512
L = F * F           # 1024
B_PER_CORE = 2
N_CORES = 8
AF = mybir.ActivationFunctionType

_cache = {}


def _build():
    nc = bacc.Bacc("TRN2", target_bir_lowering=False, debug=False,
                   num_devices=N_CORES)
    # host-packed layouts (see _prep_inputs)
    xin = nc.dram_tensor("xin", [B_PER_CORE, 8, 128, 4 * 128], BF16,
                         kind="ExternalInput").ap()
    wqt = nc.dram_tensor("wqt", [128, 4 * DIM], BF16, kind="ExternalInput").ap()
    wkt = nc.dram_tensor("wkt", [128, 4 * DIM], BF16, kind="ExternalInput").ap()
    wvt = nc.dram_tensor("wvt", [128, 4 * DIM], BF16, kind="ExternalInput").ap()
    relwt = nc.dram_tensor("relwt", [128, 64], BF16, kind="ExternalInput").ap()
    relht = nc.dram_tensor("relht", [128, 64], BF16, kind="ExternalInput").ap()
    sel = nc.dram_tensor("sel", [128, 8 * 128], BF16, kind="ExternalInput").ap()
    ones = nc.dram_tensor("ones", [128, 128], BF16, kind="ExternalInput").ap()
    ident = nc.dram_tensor("ident", [128, 128], BF16, kind="ExternalInput").ap()
    out = nc.dram_tensor("out", [B_PER_CORE, DIM, L], FP32, kind="ExternalOutput").ap()

    from contextlib import ExitStack
    ctx = ExitStack()
    with tile.TileContext(nc) as tc, ctx:
        consts = ctx.enter_context(tc.tile_pool(name="consts", bufs=1))
        xpool = ctx.enter_context(tc.tile_pool(name="xpool", bufs=2))
        vtpool = ctx.enter_context(tc.tile_pool(name="vtpool", bufs=2))
        qkpool = ctx.enter_context(tc.tile_pool(name="qkpool", bufs=2))
        rwpool = ctx.enter_context(tc.tile_pool(name="rwpool", bufs=2))
        biaspool = ctx.enter_context(tc.tile_pool(name="biaspool", bufs=2))
        ptpool = ctx.enter_context(tc.tile_pool(name="ptpool", bufs=2))
        outpool = ctx.enter_context(tc.tile_pool(name="outpool", bufs=2))
        # PSUM: 8 banks total: st 3 + attn 2 + sums 1 + misc 2
        stps = ctx.enter_context(tc.tile_pool(name="stps", bufs=3, space="PSUM"))
        attnps = ctx.enter_context(tc.tile_pool(name="attnps", bufs=2, space="PSUM"))
        sumsps = ctx.enter_context(tc.tile_pool(name="sumsps", bufs=1, space="PSUM"))
        miscps = ctx.enter_context(tc.tile_pool(name="miscps", bufs=2, space="PSUM"))
        dramw = ctx.enter_context(tc.tile_pool(name="dramw", bufs=2, space="DRAM"))
        dramh = ctx.enter_context(tc.tile_pool(name="dramh", bufs=2, space="DRAM"))

        # ---- constants ride the scalar HWDGE queue so they issue in
        # parallel with x/bounce/output traffic on the sync queue ----
        def cload(ap, shape, tag):
            t = consts.tile(shape, ap.dtype, tag=tag)
            nc.scalar.dma_start(t[:], ap)
            return t

        x_sb = [None, None]

        def emit_xload(b):
            # x SBUF free layout is (yt, c, 128): per-yt loads are contiguous
            # on both sides, so V-proj can start after the first fast 128KB
            x_sb[b] = xpool.tile([128, 8, 4, 128], BF16, tag="x", name=f"x{b}")
            for yt in range(8):
                nc.sync.dma_start(x_sb[b][:, yt], xin[b, yt])

        wv_sb = cload(wvt, [128, 4 * DIM], "wv")
        emit_xload(0)
        wq_sb = cload(wqt, [128, 4 * DIM], "wq")
        wk_sb = cload(wkt, [128, 4 * DIM], "wk")
        relw_sb = cload(relwt, [128, 64], "relw")
        relh_sb = cload(relht, [128, 64], "relh")
        id_sb = cload(ident, [128, 128], "ident")
        sel_sb = cload(sel, [128, 8 * 128], "sel")
        ones_sb = cload(ones, [128, 128], "ones")

        # per-batch state
        vt_sb = [[None] * 8, [None] * 8]
        qs = [[None] * HEADS, [None] * HEADS]
        ks = [[None] * HEADS, [None] * HEADS]
        biases = [[None] * HEADS, [None] * HEADS]

        def xc(b, c, lo, hi):
            # x chunk c, L-columns [lo:hi) (lo/hi multiples of 128)
            return x_sb[b][:, lo // 128: hi // 128, c, :]

        def emit_vproj(b, yts):
            for yt in yts:
                ps = miscps.tile([128, DIM], FP32, tag="misc")
                for c in range(4):
                    nc.tensor.matmul(ps[:], xc(b, c, yt * 128, (yt + 1) * 128),
                                     wv_sb[:, c * DIM:(c + 1) * DIM],
                                     start=(c == 0), stop=(c == 3))
                vt = vtpool.tile([128, DIM], BF16, tag=f"vt{yt}")
                nc.vector.tensor_copy(vt[:], ps[:])
                vt_sb[b][yt] = vt

        def emit_qkproj(b, h):
            q_sb = qkpool.tile([128, L], BF16, tag=f"q{h}")
            k_sb = qkpool.tile([128, L], BF16, tag=f"k{h}")
            qs[b][h] = q_sb
            ks[b][h] = k_sb
            for dst, w in ((q_sb, wq_sb), (k_sb, wk_sb)):
                pss = [miscps.tile([128, 512], FP32, tag="misc", name=f"qkps{i}")
                       for i in range(2)]
                for c in range(4):
                    for n in range(2):
                        nc.tensor.matmul(pss[n][:],
                                         w[:, c * DIM + h * 128: c * DIM + (h + 1) * 128],
                                         xc(b, c, n * 512, (n + 1) * 512),
                                         start=(c == 0), stop=(c == 3))
                for n in range(2):
                    nc.vector.tensor_copy(dst[:, n * 512:(n + 1) * 512], pss[n][:])

        wst_tiles = [[None] * 8, [None] * 8]

        def emit_rel_a(b, h):
            # matmuls + DRAM-bounce DMA issues; no PE instruction here waits
            # on the bounce, so the PE queue never stalls on DMA latency
            q_sb = qs[b][h]
            # [128, L] with rows 64+ zeroed: the bias matmul then runs at the
            # same K=128 PE tile config as every other matmul (the 128<->64
            # config switch costs ~120ns per matmul); sel rows 64+ are zero
            # so the padding rows never contribute
            bias_rhs = biaspool.tile([128, L], BF16, tag=f"bias{h}")
            biases[b][h] = bias_rhs
            nc.gpsimd.memset(bias_rhs[64:128, :], 0)
            # ---- raw width logits rl[q, m] = q . relw, all 8 q-blocks in 1 bank
            rwps = miscps.tile([128, 512], FP32, tag="misc")
            for j in range(8):
                nc.tensor.matmul(rwps[:, j * 64:(j + 1) * 64],
                                 q_sb[:, j * 128:(j + 1) * 128], relw_sb[:],
                                 start=True, stop=True)
            rwall = rwpool.tile([128, 512], BF16, tag="rwall")
            nc.vector.tensor_copy(rwall[:], rwps[:])
            # bounce to DRAM in [q, m] row-major order (row stride 64);
            # the whole bounce rides the idle gpsimd SWDGE queue so it never
            # contends with bulk loads/stores on the sync queue
            skw = dramw.tile([L, 64], BF16, tag="skw")
            skw_flat = skw[:].flatten()
            dst = bass.AP(skw_flat.tensor, skw_flat.offset,
                          [[64, 128], [8192, 8], [1, 64]])
            nc.sync.dma_start(dst, rwall[:])
            # ---- skewed reads per 128-wide x-block j ----
            for j in range(8):
                wstj = rwpool.tile([128, 32], BF16, tag=f"wst{j}", name=f"wst{j}")
                src = bass.AP(skw_flat.tensor,
                              skw_flat.offset + 8192 * j + 31,
                              [[2048, 4], [63, 32], [1, 32]])
                nc.sync.dma_start(wstj[:], src)
                wst_tiles[b][j] = wstj
            # ---- height logits rl_h[m, q] (relht col 63 zero-padded) ----
            rh = rwpool.tile([64, L], BF16, tag="rh")
            for n in range(2):
                ps = miscps.tile([64, 512], FP32, tag="misc")
                nc.tensor.matmul(ps[:], relh_sb[:],
                                 q_sb[:, n * 512:(n + 1) * 512],
                                 start=True, stop=True)
                nc.vector.tensor_copy(rh[:, n * 512:(n + 1) * 512], ps[:])
            skh = dramh.tile([64, L], BF16, tag="skh")
            nc.sync.dma_start(skh[:], rh[:])
            skh_flat = skh[:].flatten()
            hsrc = bass.AP(skh_flat.tensor, skh_flat.offset,
                           [[1024, 32], [1056, 32], [1, 32]])
            bias_flat = bias_rhs[:]
            hdst = bass.AP(bias_flat.tensor, bias_flat.offset + 32 * 1024,
                           [[1024, 32], [32, 32], [1, 32]])
            nc.sync.dma_start(hdst, hsrc)

        def emit_rel_b(b, h):
            # transposes of the skew-read tiles into bias_rhs rows [0:32)
            bias_rhs = biases[b][h]
            for half in range(2):
                tps = miscps.tile([32, 512], BF16, tag="misc")
                for jj in range(4):
                    j = half * 4 + jj
                    nc.tensor.transpose(tps[0:32, jj * 128:(jj + 1) * 128],
                                        wst_tiles[b][j][:], id_sb[:])
                nc.vector.tensor_copy(
                    bias_rhs[0:32, half * 512:(half + 1) * 512], tps[0:32, :])

        def emit_S(b, h, n, tree=True):
            q_sb, k_sb, bias_rhs = qs[b][h], ks[b][h], biases[b][h]
            pts = []
            for yt in range(8):
                st = stps.tile([128, 512], FP32, tag="st")
                nc.tensor.matmul(st[:], k_sb[:, yt * 128:(yt + 1) * 128],
                                 q_sb[:, n * 512:(n + 1) * 512],
                                 start=True, stop=False)
                nc.tensor.matmul(st[:], sel_sb[:, yt * 128:(yt + 1) * 128],
                                 bias_rhs[:, n * 512:(n + 1) * 512],
                                 start=False, stop=True)
                pt = ptpool.tile([128, 512], BF16, tag=f"pt{yt}")
                nc.scalar.activation(pt[:], st[:], AF.Exp)
                pts.append(pt)
            if not tree:
                return pts, None
            # DVE pairwise tree for the softmax denominator: the PE then only
            # streams one ones-matmul per unit instead of 8 (bf16 ops hit the
            # DVE 2x mode; final bf16 rounding costs ~0.2% on the denominator)
            l1 = [ptpool.tile([128, 512], BF16, tag=f"l1{i}", name=f"l1{i}")
                  for i in range(4)]
            for i in range(4):
                nc.vector.tensor_add(l1[i][:], pts[2 * i][:], pts[2 * i + 1][:])
            l2 = [ptpool.tile([128, 512], BF16, tag=f"l2{i}", name=f"l2{i}")
                  for i in range(2)]
            for i in range(2):
                nc.vector.tensor_add(l2[i][:], l1[2 * i][:], l1[2 * i + 1][:])
            psum_all = ptpool.tile([128, 512], BF16, tag="psall")
            nc.vector.tensor_add(psum_all[:], l2[0][:], l2[1][:])
            return pts, psum_all

        def emit_tail(b, h, n, pts, psum_all, split=False):
            sums = sumsps.tile([128, 512], FP32, tag="sums")
            attn = attnps.tile([128, 512], FP32, tag="attn")
            if psum_all is None:
                # last unit: PE-sums before attn, so the reciprocal overlaps
                # the attn matmuls and the final-store chain is short
                for yt in range(8):
                    nc.tensor.matmul(sums[:], ones_sb[:], pts[yt][:],
                                     start=(yt == 0), stop=(yt == 7))
            for yt in range(8):
                nc.tensor.matmul(attn[:], vt_sb[b][yt][:, h * 128:(h + 1) * 128],
                                 pts[yt][:], start=(yt == 0), stop=(yt == 7))
            if psum_all is not None:
                nc.tensor.matmul(sums[:], ones_sb[:], psum_all[:],
                                 start=True, stop=True)
            recip = outpool.tile([128, 512], FP32, tag="recip")
            o_sb = outpool.tile([128, 512], FP32, tag="osb")
            # split halves on the last unit so the final store starts sooner
            for lo, hi in ([(0, 256), (256, 512)] if split else [(0, 512)]):
                nc.vector.reciprocal_approx_fast(out=recip[:, lo:hi],
                                                 in_=sums[:, lo:hi])
                nc.vector.tensor_mul(o_sb[:, lo:hi], attn[:, lo:hi],
                                     recip[:, lo:hi])
                nc.sync.dma_start(
                    out[b, h * 128:(h + 1) * 128, n * 512 + lo: n * 512 + hi],
                    o_sb[:, lo:hi])

        # ---- emission schedule ----
        # per batch: [V01, V23, QK0, RELa0, QK1, RELb0, RELa1, QK2, RELb1,
        #             RELa2, QK3, RELb2, RELa3, RELb3]; a unit (b, h) needs
        # thunks through RELb(h) => req offsets {0: 6, 1: 9, 2: 12, 3: 14}
        def phase1_thunks(b):
            ts = [lambda b=b: emit_vproj(b, range(0, 4)),
                  lambda b=b: emit_vproj(b, range(4, 8)),
                  lambda b=b: emit_qkproj(b, 0),
                  lambda b=b: emit_rel_a(b, 0)]
            for h in range(1, HEADS):
                ts += [lambda b=b, h=h: emit_qkproj(b, h),
                       lambda b=b, h=h: emit_rel_b(b, h - 1),
                       lambda b=b, h=h: emit_rel_a(b, h)]
            ts += [lambda b=b: emit_rel_b(b, HEADS - 1)]
            return ts

        REQ = {0: 6, 1: 9, 2: 12, 3: 14}
        thunks = (phase1_thunks(0) +
                  [lambda: emit_xload(1)] + phase1_thunks(1))
        n_consumed = 0

        def consume(upto=None, extra=0):
            nonlocal n_consumed
            target = n_consumed + extra if upto is None else max(upto, n_consumed)
            target = min(target, len(thunks))
            while n_consumed < target:
                thunks[n_consumed]()
                n_consumed += 1

        units = [(b, h, n) for b in range(B_PER_CORE)
                 for h in range(HEADS) for n in range(2)]
        from collections import deque
        pending = deque()
        consume(upto=7)   # fill the cold-start bounce latency with PE work
        for ui, (b, h, n) in enumerate(units):
            consume(upto=15 * b + REQ[h])   # need V + QK/REL through head h
            pts, psum_all = emit_S(b, h, n, tree=(ui < len(units) - 1))
            consume(extra=1)
            if len(pending) == 1:
                emit_tail(*pending.popleft())
            consume(extra=1)
            pending.append((b, h, n, pts, psum_all))
        while pending:
            emit_tail(*pending.popleft())

    nc.compile()
    return nc


def _prep_inputs(featuremap, w_qk, w_v, rel_height, rel_width):
    scale = D ** -0.5
    # weights packed as [128, c_chunk*512]: w[p, c*512+d] = W.T[c*128+p, d]
    def packw(wt):  # wt: [512(c), 512(d)]
        return np.ascontiguousarray(
            wt.reshape(4, 128, DIM).transpose(1, 0, 2).reshape(128, 4 * DIM)
        ).astype(bf16)
    wqt = packw(w_qk[:DIM].T * scale)
    wkt = packw(w_qk[DIM:].T)
    wvt = packw(w_v.T)
    relwt = np.zeros((128, 64), np.float32)
    relwt[:, :63] = rel_width.T
    relwt = relwt.astype(bf16)
    relht = np.zeros((128, 64), np.float32)
    relht[:, :63] = rel_height.T[:, ::-1]
    relht = relht.astype(bf16)
    yy = np.arange(128)
    sel = np.zeros((128, 8 * 128), np.float32)
    for yt in range(8):
        sel[yy % 32, yt * 128 + yy] = 1.0
        sel[32 + 31 - (yt * 4 + yy // 32), yt * 128 + yy] = 1.0
    sel = sel.astype(bf16)
    ones = np.ones((128, 128), bf16)
    ident = np.eye(128, dtype=bf16)
    common = dict(wqt=wqt, wkt=wkt, wvt=wvt, relwt=relwt, relht=relht,
                  sel=sel, ones=ones, ident=ident)
    # x packed per batch as [8(yt), 128(p), 4(c), 128(l)] (yt-major, so each
    # per-yt DMA is contiguous on both sides)
    xin = featuremap.reshape(16, 4, 128, 8, 128).transpose(0, 3, 2, 1, 4).reshape(
        N_CORES, B_PER_CORE, 8, 128, 4 * 128).astype(bf16)
    return [dict(common, xin=np.ascontiguousarray(xin[i])) for i in range(N_CORES)]


def kernel(featuremap, w_qk, w_v, rel_height, rel_width, _trace=False, _tmpdir=None):
    if "nc" not in _cache:
        _cache["nc"] = _build()
    nc = _cache["nc"]
    in_maps = _prep_inputs(featuremap, w_qk, w_v, rel_height, rel_width)
    res = run_bass_kernel_spmd(nc, in_maps, list(range(N_CORES)),
                               trace=_trace, tmpdir=_tmpdir)
    _cache["last_result"] = res
    full = np.concatenate([res.results[i]["out"] for i in range(N_CORES)], axis=0)
    return full.reshape(16, DIM, F, F)


# revision 61
# speedup vs baseline: 1.1765x; 1.0268x over previous
"""Trainium2 Bass kernel for nn_MHSA_37821482008969 (2D rel-pos MHSA).

Strategy: data-parallel over batch (16 batches -> 8 cores x 2). Per (batch,
head) unit, attention is computed fully transposed: S^T = K^T@Q tiles with
y (keys) on partitions, so softmax-normalization sums come from a ones-matrix
matmul on PE (replicated across all 128 partitions, so the reciprocal and
final scale run as plain full-width DVE ops), the attn matmul needs no
transposes of exp(S), and the output lands directly in the channel-major
layout the conv output wants.

Rel-pos biases are folded into the logits accumulation as one extra K=64
matmul per tile: lhsT is a constant 0/1 selector, rhs is the skewed rel-logit
table built via a DRAM round-trip (regular strided APs implement the
rel->abs skew) plus small per-j-block PE transposes for the width term.

All matmul operands are bf16 (fp32 PSUM accumulation); softmax skips the
row-max subtraction (logits are ~N(0,1), |logit| < 7, exp is safe in fp32).

The emission order software-pipelines the attention units (S-matmuls of unit
i+1 before the normalization tail of unit i) and interleaves batch 1's
projection/bias phase into batch 0's attention phase so the PE never idles
(keeps the tensor engine p-state at max clock).
"""
import numpy as np
import ml_dtypes

import concourse.bass as bass
import concourse.mybir as mybir
import concourse.tile as tile
import concourse.bacc as bacc
from concourse.bass_utils import run_bass_kernel_spmd

bf16 = ml_dtypes.bfloat16
f8e4 = ml_dtypes.float8_e4m3
FP32 = mybir.dt.float32
BF16 = mybir.dt.bfloat16
F8E4 = mybir.dt.float8e4
DR = mybir.MatmulPerfMode.DoubleRow

HEADS, D, F, DIM = 4, 128, 32, 512
L = F * F           # 1024
B_PER_CORE = 2
N_CORES = 8
AF = mybir.ActivationFunctionType

_cache = {}


def _build():
    nc = bacc.Bacc("TRN2", target_bir_lowering=False, debug=False,
                   num_devices=N_CORES)
    # host-packed layouts (see _prep_inputs)
    xin = nc.dram_tensor("xin", [B_PER_CORE, 8, 128, 4 * 128], BF16,
                         kind="ExternalInput").ap()
    wqt = nc.dram_tensor("wqt", [128, 4 * DIM], BF16, kind="ExternalInput").ap()
    wkt = nc.dram_tensor("wkt", [128, 4 * DIM], BF16, kind="ExternalInput").ap()
    wvt = nc.dram_tensor("wvt", [128, 4 * DIM], BF16, kind="ExternalInput").ap()
    relwt = nc.dram_tensor("relwt", [128, 64], BF16, kind="ExternalInput").ap()
    relht = nc.dram_tensor("relht", [128, 128], BF16, kind="ExternalInput").ap()
    sel = nc.dram_tensor("sel", [128, 8 * 128], BF16, kind="ExternalInput").ap()
    ones = nc.dram_tensor("ones", [128, 128], BF16, kind="ExternalInput").ap()
    ident = nc.dram_tensor("ident", [128, 128], BF16, kind="ExternalInput").ap()
    out = nc.dram_tensor("out", [B_PER_CORE, DIM, L], FP32, kind="ExternalOutput").ap()

    from contextlib import ExitStack
    ctx = ExitStack()
    with tile.TileContext(nc) as tc, ctx:
        consts = ctx.enter_context(tc.tile_pool(name="consts", bufs=1))
        xpool = ctx.enter_context(tc.tile_pool(name="xpool", bufs=2))
        vtpool = ctx.enter_context(tc.tile_pool(name="vtpool", bufs=2))
        qkpool = ctx.enter_context(tc.tile_pool(name="qkpool", bufs=2))
        rwpool = ctx.enter_context(tc.tile_pool(name="rwpool", bufs=2))
        biaspool = ctx.enter_context(tc.tile_pool(name="biaspool", bufs=2))
        ptpool = ctx.enter_context(tc.tile_pool(name="ptpool", bufs=2))
        outpool = ctx.enter_context(tc.tile_pool(name="outpool", bufs=2))
        # PSUM: 8 banks total: st 3 + attn 2 + sums 1 + misc 2
        stps = ctx.enter_context(tc.tile_pool(name="stps", bufs=3, space="PSUM"))
        attnps = ctx.enter_context(tc.tile_pool(name="attnps", bufs=2, space="PSUM"))
        sumsps = ctx.enter_context(tc.tile_pool(name="sumsps", bufs=1, space="PSUM"))
        miscps = ctx.enter_context(tc.tile_pool(name="miscps", bufs=2, space="PSUM"))
        dramw = ctx.enter_context(tc.tile_pool(name="dramw", bufs=2, space="DRAM"))
        dramh = ctx.enter_context(tc.tile_pool(name="dramh", bufs=2, space="DRAM"))

        # ---- constants ride the scalar HWDGE queue so they issue in
        # parallel with x/bounce/output traffic on the sync queue ----
        def cload(ap, shape, tag):
            t = consts.tile(shape, ap.dtype, tag=tag)
            nc.scalar.dma_start(t[:], ap)
            return t

        x_sb = [None, None]

        def emit_xload(b):
            # x SBUF free layout is (yt, c, 128): per-yt loads are contiguous
            # on both sides, so V-proj can start after the first fast 128KB;
            # chunks alternate between the two HWDGE queues to double the
            # issue rate (V-proj otherwise outruns one queue's ~0.6us/issue)
            x_sb[b] = xpool.tile([128, 8, 4, 128], BF16, tag="x", name=f"x{b}")
            for yt in range(8):
                eng = nc.sync if yt % 2 == 0 else nc.scalar
                eng.dma_start(x_sb[b][:, yt], xin[b, yt])

        wv_sb = cload(wvt, [128, 4 * DIM], "wv")
        emit_xload(0)
        wq_sb = cload(wqt, [128, 4 * DIM], "wq")
        wk_sb = cload(wkt, [128, 4 * DIM], "wk")
        relw_sb = cload(relwt, [128, 64], "relw")
        relh_sb = cload(relht, [128, 128], "relh")
        id_sb = cload(ident, [128, 128], "ident")
        sel_sb = cload(sel, [128, 8 * 128], "sel")
        ones_sb = cload(ones, [128, 128], "ones")

        # per-batch state
        vt_sb = [[None] * 8, [None] * 8]
        qs = [[None] * HEADS, [None] * HEADS]
        ks = [[None] * HEADS, [None] * HEADS]
        biases = [[None] * HEADS, [None] * HEADS]

        def xc(b, c, lo, hi):
            # x chunk c, L-columns [lo:hi) (lo/hi multiples of 128)
            return x_sb[b][:, lo // 128: hi // 128, c, :]

        def emit_vproj(b, yts):
            for yt in yts:
                ps = miscps.tile([128, DIM], FP32, tag="misc")
                for c in range(4):
                    nc.tensor.matmul(ps[:], xc(b, c, yt * 128, (yt + 1) * 128),
                                     wv_sb[:, c * DIM:(c + 1) * DIM],
                                     start=(c == 0), stop=(c == 3))
                vt = vtpool.tile([128, DIM], BF16, tag=f"vt{yt}")
                nc.vector.tensor_copy(vt[:], ps[:])
                vt_sb[b][yt] = vt

        def emit_qkproj(b, h):
            q_sb = qkpool.tile([128, L], BF16, tag=f"q{h}")
            k_sb = qkpool.tile([128, L], BF16, tag=f"k{h}")
            qs[b][h] = q_sb
            ks[b][h] = k_sb
            for dst, w in ((q_sb, wq_sb), (k_sb, wk_sb)):
                pss = [miscps.tile([128, 512], FP32, tag="misc", name=f"qkps{i}")
                       for i in range(2)]
                for c in range(4):
                    for n in range(2):
                        nc.tensor.matmul(pss[n][:],
                                         w[:, c * DIM + h * 128: c * DIM + (h + 1) * 128],
                                         xc(b, c, n * 512, (n + 1) * 512),
                                         start=(c == 0), stop=(c == 3))
                for n in range(2):
                    nc.vector.tensor_copy(dst[:, n * 512:(n + 1) * 512], pss[n][:])

        wst_tiles = [[None] * 8, [None] * 8]

        def emit_rel_a(b, h):
            # matmuls + DRAM-bounce DMA issues; no PE instruction here waits
            # on the bounce, so the PE queue never stalls on DMA latency
            q_sb = qs[b][h]
            # [128, L] with rows 64+ zeroed: the bias matmul then runs at the
            # same K=128 PE tile config as every other matmul (the 128<->64
            # config switch costs ~120ns per matmul); sel rows 64+ are zero
            # so the padding rows never contribute
            bias_rhs = biaspool.tile([128, L], BF16, tag=f"bias{h}")
            biases[b][h] = bias_rhs
            nc.gpsimd.memset(bias_rhs[64:128, :], 0)
            # ---- raw width logits rl[q, m] = q . relw, all 8 q-blocks in 1 bank
            rwps = miscps.tile([128, 512], FP32, tag="misc")
            for j in range(8):
                nc.tensor.matmul(rwps[:, j * 64:(j + 1) * 64],
                                 q_sb[:, j * 128:(j + 1) * 128], relw_sb[:],
                                 start=True, stop=True)
            rwall = rwpool.tile([128, 512], BF16, tag="rwall")
            nc.vector.tensor_copy(rwall[:], rwps[:])
            # bounce to DRAM in [q, m] row-major order (row stride 64);
            # the whole bounce rides the idle gpsimd SWDGE queue so it never
            # contends with bulk loads/stores on the sync queue
            skw = dramw.tile([L, 64], BF16, tag="skw")
            skw_flat = skw[:].flatten()
            dst = bass.AP(skw_flat.tensor, skw_flat.offset,
                          [[64, 128], [8192, 8], [1, 64]])
            nc.sync.dma_start(dst, rwall[:])
            # ---- skewed reads per 128-wide x-block j (tiles padded to 128
            # free so the transposes keep the 128-col PE config; cols 32+
            # are stale garbage that the transpose moves to never-read rows)
            for j in range(8):
                wstj = rwpool.tile([128, 128], BF16, tag=f"wst{j}", name=f"wst{j}")
                src = bass.AP(skw_flat.tensor,
                              skw_flat.offset + 8192 * j + 31,
                              [[2048, 4], [63, 32], [1, 32]])
                nc.sync.dma_start(wstj[:, 0:32], src)
                wst_tiles[b][j] = wstj
            # ---- height logits rl_h[m, q] (relht cols 63+ zero-padded so
            # the matmul keeps the M=128 PE col config) ----
            rh = rwpool.tile([64, L], BF16, tag="rh")
            for n in range(2):
                ps = miscps.tile([128, 512], FP32, tag="misc")
                nc.tensor.matmul(ps[:], relh_sb[:],
                                 q_sb[:, n * 512:(n + 1) * 512],
                                 start=True, stop=True)
                nc.vector.tensor_copy(rh[:, n * 512:(n + 1) * 512], ps[0:64, :])
            skh = dramh.tile([64, L], BF16, tag="skh")
            nc.sync.dma_start(skh[:], rh[:])
            skh_flat = skh[:].flatten()
            hsrc = bass.AP(skh_flat.tensor, skh_flat.offset,
                           [[1024, 32], [1056, 32], [1, 32]])
            bias_flat = bias_rhs[:]
            hdst = bass.AP(bias_flat.tensor, bias_flat.offset + 32 * 1024,
                           [[1024, 32], [32, 32], [1, 32]])
            nc.sync.dma_start(hdst, hsrc)

        def emit_rel_b(b, h):
            # transposes of the skew-read tiles into bias_rhs rows [0:32)
            bias_rhs = biases[b][h]
            for half in range(2):
                tps = miscps.tile([128, 512], BF16, tag="misc")
                for jj in range(4):
                    j = half * 4 + jj
                    nc.tensor.transpose(tps[:, jj * 128:(jj + 1) * 128],
                                        wst_tiles[b][j][:], id_sb[:])
                nc.vector.tensor_copy(
                    bias_rhs[0:32, half * 512:(half + 1) * 512], tps[0:32, :])

        def emit_S(b, h, n, tree=True):
            q_sb, k_sb, bias_rhs = qs[b][h], ks[b][h], biases[b][h]
            pts = []
            for yt in range(8):
                st = stps.tile([128, 512], FP32, tag="st")
                nc.tensor.matmul(st[:], k_sb[:, yt * 128:(yt + 1) * 128],
                                 q_sb[:, n * 512:(n + 1) * 512],
                                 start=True, stop=False)
                nc.tensor.matmul(st[:], sel_sb[:, yt * 128:(yt + 1) * 128],
                                 bias_rhs[:, n * 512:(n + 1) * 512],
                                 start=False, stop=True)
                pt = ptpool.tile([128, 512], BF16, tag=f"pt{yt}")
                nc.scalar.activation(pt[:], st[:], AF.Exp)
                pts.append(pt)
            if not tree:
                return pts, None
            # DVE pairwise tree for the softmax denominator: the PE then only
            # streams one ones-matmul per unit instead of 8 (bf16 ops hit the
            # DVE 2x mode; final bf16 rounding costs ~0.2% on the denominator)
            l1 = [ptpool.tile([128, 512], BF16, tag=f"l1{i}", name=f"l1{i}")
                  for i in range(4)]
            for i in range(4):
                nc.vector.tensor_add(l1[i][:], pts[2 * i][:], pts[2 * i + 1][:])
            l2 = [ptpool.tile([128, 512], BF16, tag=f"l2{i}", name=f"l2{i}")
                  for i in range(2)]
            for i in range(2):
                nc.vector.tensor_add(l2[i][:], l1[2 * i][:], l1[2 * i + 1][:])
            psum_all = ptpool.tile([128, 512], BF16, tag="psall")
            nc.vector.tensor_add(psum_all[:], l2[0][:], l2[1][:])
            return pts, psum_all

        def emit_tail(b, h, n, pts, psum_all, split=False):
            sums = sumsps.tile([128, 512], FP32, tag="sums")
            attn = attnps.tile([128, 512], FP32, tag="attn")
            if psum_all is None:
                # last unit: PE-sums before attn, so the reciprocal overlaps
                # the attn matmuls and the final-store chain is short
                for yt in range(8):
                    nc.tensor.matmul(sums[:], ones_sb[:], pts[yt][:],
                                     start=(yt == 0), stop=(yt == 7))
            for yt in range(8):
                nc.tensor.matmul(attn[:], vt_sb[b][yt][:, h * 128:(h + 1) * 128],
                                 pts[yt][:], start=(yt == 0), stop=(yt == 7))
            if psum_all is not None:
                nc.tensor.matmul(sums[:], ones_sb[:], psum_all[:],
                                 start=True, stop=True)
            recip = outpool.tile([128, 512], FP32, tag="recip")
            o_sb = outpool.tile([128, 512], FP32, tag="osb")
            # split halves on the last unit so the final store starts sooner
            for lo, hi in ([(0, 256), (256, 512)] if split else [(0, 512)]):
                nc.vector.reciprocal_approx_fast(out=recip[:, lo:hi],
                                                 in_=sums[:, lo:hi])
                nc.vector.tensor_mul(o_sb[:, lo:hi], attn[:, lo:hi],
                                     recip[:, lo:hi])
                nc.sync.dma_start(
                    out[b, h * 128:(h + 1) * 128, n * 512 + lo: n * 512 + hi],
                    o_sb[:, lo:hi])

        # ---- emission schedule ----
        # per batch: [V01, V23, QK0, RELa0, QK1, RELb0, RELa1, QK2, RELb1,
        #             RELa2, QK3, RELb2, RELa3, RELb3]; a unit (b, h) needs
        # thunks through RELb(h) => req offsets {0: 6, 1: 9, 2: 12, 3: 14}
        def phase1_thunks(b):
            ts = [lambda b=b: emit_vproj(b, range(0, 4)),
                  lambda b=b: emit_vproj(b, range(4, 8)),
                  lambda b=b: emit_qkproj(b, 0),
                  lambda b=b: emit_rel_a(b, 0)]
            for h in range(1, HEADS):
                ts += [lambda b=b, h=h: emit_qkproj(b, h),
                       lambda b=b, h=h: emit_rel_b(b, h - 1),
                       lambda b=b, h=h: emit_rel_a(b, h)]
            ts += [lambda b=b: emit_rel_b(b, HEADS - 1)]
            return ts

        REQ = {0: 6, 1: 9, 2: 12, 3: 14}
        thunks = (phase1_thunks(0) +
                  [lambda: emit_xload(1)] + phase1_thunks(1))
        n_consumed = 0

        def consume(upto=None, extra=0):
            nonlocal n_consumed
            target = n_consumed + extra if upto is None else max(upto, n_consumed)
            target = min(target, len(thunks))
            while n_consumed < target:
                thunks[n_consumed]()
                n_consumed += 1

        units = [(b, h, n) for b in range(B_PER_CORE)
                 for h in range(HEADS) for n in range(2)]
        from collections import deque
        pending = deque()
        consume(upto=7)   # fill the cold-start bounce latency with PE work
        for ui, (b, h, n) in enumerate(units):
            consume(upto=15 * b + REQ[h])   # need V + QK/REL through head h
            pts, psum_all = emit_S(b, h, n, tree=(ui < len(units) - 1))
            consume(extra=1)
            if len(pending) == 1:
                emit_tail(*pending.popleft())
            consume(extra=1)
            pending.append((b, h, n, pts, psum_all))
        while pending:
            emit_tail(*pending.popleft())

    nc.compile()
    return nc


def _prep_inputs(featuremap, w_qk, w_v, rel_height, rel_width):
    scale = D ** -0.5
    # weights packed as [128, c_chunk*512]: w[p, c*512+d] = W.T[c*128+p, d]
    def packw(wt):  # wt: [512(c), 512(d)]
        return np.ascontiguousarray(
            wt.reshape(4, 128, DIM).transpose(1, 0, 2).reshape(128, 4 * DIM)
        ).astype(bf16)
    wqt = packw(w_qk[:DIM].T * scale)
    wkt = packw(w_qk[DIM:].T)
    wvt = packw(w_v.T)
    relwt = np.zeros((128, 64), np.float32)
    relwt[:, :63] = rel_width.T
    relwt = relwt.astype(bf16)
    relht = np.zeros((128, 128), np.float32)
    relht[:, :63] = rel_height.T[:, ::-1]
    relht = relht.astype(bf16)
    yy = np.arange(128)
    sel = np.zeros((128, 8 * 128), np.float32)
    for yt in range(8):
        sel[yy % 32, yt * 128 + yy] = 1.0
        sel[32 + 31 - (yt * 4 + yy // 32), yt * 128 + yy] = 1.0
    sel = sel.astype(bf16)
    ones = np.ones((128, 128), bf16)
    ident = np.eye(128, dtype=bf16)
    common = dict(wqt=wqt, wkt=wkt, wvt=wvt, relwt=relwt, relht=relht,
                  sel=sel, ones=ones, ident=ident)
    # x packed per batch as [8(yt), 128(p), 4(c), 128(l)] (yt-major, so each
    # per-yt DMA is contiguous on both sides)
    xin = featuremap.reshape(16, 4, 128, 8, 128).transpose(0, 3, 2, 1, 4).reshape(
        N_CORES, B_PER_CORE, 8, 128, 4 * 128).astype(bf16)
    return [dict(common, xin=np.ascontiguousarray(xin[i])) for i in range(N_CORES)]


def kernel(featuremap, w_qk, w_v, rel_height, rel_width, _trace=False, _tmpdir=None):
    if "nc" not in _cache:
        _cache["nc"] = _build()
    nc = _cache["nc"]
    in_maps = _prep_inputs(featuremap, w_qk, w_v, rel_height, rel_width)
    res = run_bass_kernel_spmd(nc, in_maps, list(range(N_CORES)),
                               trace=_trace, tmpdir=_tmpdir)
    _cache["last_result"] = res
    full = np.concatenate([res.results[i]["out"] for i in range(N_CORES)], axis=0)
    return full.reshape(16, DIM, F, F)
